# revision 10
# baseline (speedup 1.0000x reference)
"""JacobianDeterminantLoss Trainium2 kernel (8-core SPMD).

Math: u [2,3,160,192,160] f32 -> loss = mean(relu(-det(J))) where
J = I + grad(phi), phi_c = u_c * (dim_c-1)/2, gradients np.gradient
style (central interior, one-sided edges; ghosts 2a-b make both
uniform central diffs).

Layout (per core): core = (batch b, D-quarter q). Partitions =
3 H-slots x 42 planes (40 real + 1 halo each side) = 126. Per
partition free = 66 stored h-rows (64 real + halo) x 162 cols
(160 + ghost) fp16.

Host folds everything linear into the input: phi' = u*scale/2 +
0.5*(axis_idx - center) per channel. The central diff of the ramp
is exactly the +1 diagonal of J, so the device never adds 1.

Device per 12-row chunk:
- PE: block-diag band matmul -> D-diffs A,d,g in PSUM f32.
- ACT: PSUM -> SBUF fp16 copies; final relu + free-dim accum.
- DVE: shifted-diff subs + cofactor products (fp16 2x mode).
- Pool(GPSIMD): 2 diffs + the 3 (A,d,g)*minor products as
  scalar_tensor_tensor (its cheapest op class in the cost model).
Host: mask halo partitions, sum, divide by N.
"""
import sys
import numpy as np

if '/opt/trn_rl_repo' not in sys.path:
    sys.path.insert(0, '/opt/trn_rl_repo')

B, C, D, H, W = 2, 3, 160, 192, 160
N_CORES = 8
QP = D // 4                  # 40 planes per quarter
SLOT = QP + 2                # 42 partitions per slot
NSLOT = 3
NPART = NSLOT * SLOT         # 126
RS = H // NSLOT              # 64 real rows per slot
RSTORE = RS + 2              # 66 stored rows
WG = W + 2                   # 162 stored cols
CHUNKS = [(0, 4), (4, 12), (16, 12), (28, 12), (40, 12), (52, 8), (60, 4)]
NCHUNK = len(CHUNKS)
# DMA row sections (cover chunk c + halo before chunk c computes)
SECS = [(0, 6), (6, 12), (18, 12), (30, 12), (42, 12), (54, 12)]
# chunks whose M1 f*h product runs on Pool instead of DVE (balance knob)
POOL_EXTRA = {3, 5}

_prog_cache = {}


def _build_program():
    import concourse.tile as tile
    import concourse.mybir as mybir
    from concourse import bacc

    fp16 = mybir.dt.float16
    f32 = mybir.dt.float32
    AT = mybir.AluOpType
    AF = mybir.ActivationFunctionType

    nc = bacc.Bacc("TRN2", target_bir_lowering=False, debug=False,
                   num_devices=N_CORES)
    slab_in = nc.dram_tensor("slab", [C, NPART, RSTORE, WG], fp16,
                             kind="ExternalInput")
    band_in = nc.dram_tensor("band", [128, 128], fp16, kind="ExternalInput")
    acc_out = nc.dram_tensor("acc", [NPART, NCHUNK], f32,
                             kind="ExternalOutput")

    with tile.TileContext(nc) as tc:
        with tc.tile_pool(name="inp", bufs=1) as inp, \
             tc.tile_pool(name="piece", bufs=2) as piece, \
             tc.tile_pool(name="dveonly", bufs=1) as dv, \
             tc.tile_pool(name="cross", bufs=2) as cx, \
             tc.tile_pool(name="misc", bufs=1) as misc, \
             tc.tile_pool(name="psum", bufs=1, space="PSUM") as psum:
            band = misc.tile([128, 128], fp16)
            nc.sync.dma_start(band[:], band_in[:])
            acc_sb = misc.tile([128, NCHUNK], f32)

            XYZ = []
            for c in range(C):
                t = inp.tile([128, RSTORE, WG], fp16, tag=f"in{c}")
                XYZ.append(t)
            for (s0, sn) in SECS:
                for c in range(C):
                    nc.sync.dma_start(XYZ[c][0:NPART, s0:s0 + sn],
                                      slab_in[c, :, s0:s0 + sn])
            X, Y, Z = XYZ

            def stage1(ci):
                """PE D-diffs + ACT copies + diffs/products/minors."""
                r0, nr = CHUNKS[ci]
                pcs = []
                for ch in range(C):
                    pc = piece.tile([128, 12, W], fp16, tag=f"pc{ch}",
                                    name=f"pc{ch}")
                    for g0 in range(0, nr, 6):
                        gn = min(6, nr - g0)
                        hb = (gn + 1) // 2
                        ps = psum.tile([128, 2, 512], f32, tag=f"ps{ch}",
                                       name=f"ps{ch}")
                        for hh in range(2):
                            rr = g0 + hb * hh
                            rows = min(hb, gn - hb * hh)
                            if rows <= 0:
                                continue
                            nc.tensor.matmul(
                                ps[0:NPART, hh, 0:rows * W],
                                band[0:NPART, 0:NPART],
                                XYZ[ch][0:NPART, 1 + r0 + rr:1 + r0 + rr + rows,
                                        1:1 + W],
                                start=True, stop=True)
                        nc.scalar.copy(pc[0:NPART, g0:g0 + gn, :],
                                       ps[0:NPART, :, 0:hb * W])
                    pcs.append(pc)

                def hv(t, dr):
                    return t[0:NPART, r0 + 1 + dr:r0 + 1 + dr + nr, 1:1 + W]

                def wv(t, dw):
                    return t[0:NPART, r0 + 1:r0 + 1 + nr, 1 + dw:1 + dw + W]

                def dtile(tag):
                    return dv.tile([128, 12, W], fp16, tag=tag, name=tag)

                def ctile(tag):
                    return cx.tile([128, 12, W], fp16, tag=tag, name=tag)

                def vb(t):
                    return t[0:NPART, 0:nr]

                # diffs: b on Pool; c,E,f,h_,I on DVE
                b_ = ctile("b")
                nc.gpsimd.tensor_sub(vb(b_), hv(X, 1), hv(X, -1))
                c_ = dtile("c")
                nc.vector.tensor_sub(vb(c_), wv(X, 1), wv(X, -1))
                E_ = dtile("E")
                nc.vector.tensor_sub(vb(E_), hv(Y, 1), hv(Y, -1))
                f_ = dtile("f")
                nc.vector.tensor_sub(vb(f_), wv(Y, 1), wv(Y, -1))
                h_ = dtile("h")
                nc.vector.tensor_sub(vb(h_), hv(Z, 1), hv(Z, -1))
                I_ = dtile("i")
                nc.vector.tensor_sub(vb(I_), wv(Z, 1), wv(Z, -1))

                # det = A*(EI - fh) - d*(bI - ch) + g*(bf - cE)
                p1 = dtile("p1")
                p2 = ctile("p2") if ci in POOL_EXTRA else dtile("p2")
                M1 = ctile("M1")
                nc.vector.tensor_mul(vb(p1), vb(E_), vb(I_))
                if ci in POOL_EXTRA:
                    nc.gpsimd.tensor_mul(vb(p2), vb(f_), vb(h_))
                else:
                    nc.vector.tensor_mul(vb(p2), vb(f_), vb(h_))
                nc.vector.tensor_sub(vb(M1), vb(p1), vb(p2))
                M2 = ctile("M2")
                nc.vector.tensor_mul(vb(p1), vb(b_), vb(I_))
                nc.vector.tensor_mul(vb(p2), vb(c_), vb(h_))
                nc.vector.tensor_sub(vb(M2), vb(p1), vb(p2))
                M3 = ctile("M3")
                nc.vector.tensor_mul(vb(p1), vb(b_), vb(f_))
                nc.vector.tensor_mul(vb(p2), vb(c_), vb(E_))
                nc.vector.tensor_sub(vb(M3), vb(p1), vb(p2))
                return pcs, (M1, M2, M3)

            def stage2(ci, pcs, Ms):
                r0, nr = CHUNKS[ci]
                A_, d_, g_ = pcs
                M1, M2, M3 = Ms

                def dtile(tag):
                    return dv.tile([128, 12, W], fp16, tag=tag, name=tag)

                def ctile(tag):
                    return cx.tile([128, 12, W], fp16, tag=tag, name=tag)

                def vb(t):
                    return t[0:NPART, 0:nr]

                T1 = ctile("T1")
                nc.gpsimd.tensor_mul(vb(T1), A_[0:NPART, 0:nr], vb(M1))
                T2 = ctile("T2")
                nc.gpsimd.tensor_mul(vb(T2), d_[0:NPART, 0:nr], vb(M2))
                T3 = ctile("T3")
                nc.gpsimd.tensor_mul(vb(T3), g_[0:NPART, 0:nr], vb(M3))
                n1 = dtile("n1")
                nc.vector.tensor_sub(vb(n1), vb(T2), vb(T1))
                nd = ctile("nd")
                nc.vector.tensor_sub(vb(nd), vb(n1), vb(T3))
                trash = ctile("trash")
                nc.scalar.activation(vb(trash), vb(nd), AF.Relu,
                                     accum_out=acc_sb[0:NPART, ci:ci + 1])

            pending = None
            for ci in range(NCHUNK):
                s1 = stage1(ci)
                if pending is not None:
                    stage2(ci - 1, *pending)
                pending = s1
            nc.sync.dma_start(acc_out[:, 0:NCHUNK - 1],
                              acc_sb[0:NPART, 0:NCHUNK - 1])
            stage2(NCHUNK - 1, *pending)
            nc.sync.dma_start(acc_out[:, NCHUNK - 1:NCHUNK],
                              acc_sb[0:NPART, NCHUNK - 1:NCHUNK])
    nc.compile()
    return nc


def _make_band():
    band = np.zeros((128, 128), dtype=np.float16)
    for p in range(NPART):
        j = p % SLOT
        if j <= SLOT - 2:
            band[p + 1, p] = 1.0
        if j >= 1:
            band[p - 1, p] = -1.0
    return band


def _make_slabs(u):
    """u [2,3,160,192,160] -> 8 per-core slabs [3, 126, 66, 162] fp16."""
    u = np.asarray(u, dtype=np.float32)
    sc = np.array([(D - 1) / 4.0, (H - 1) / 4.0, (W - 1) / 4.0],
                  dtype=np.float32)
    phi = u * sc[None, :, None, None, None]
    # +1 diagonal as linear ramps (centered to limit fp16 magnitude)
    rd = 0.5 * (np.arange(D, dtype=np.float32) - (D - 1) / 2.0)
    rh = 0.5 * (np.arange(H, dtype=np.float32) - (H - 1) / 2.0)
    rw = 0.5 * (np.arange(W, dtype=np.float32) - (W - 1) / 2.0)
    phi[:, 0] += rd[:, None, None]
    phi[:, 1] += rh[None, :, None]
    phi[:, 2] += rw[None, None, :]
    # pad with linear-extrapolation ghosts on all three axes
    P = np.empty((B, C, D + 2, H + 2, W + 2), dtype=np.float32)
    P[:, :, 1:D + 1, 1:H + 1, 1:W + 1] = phi
    P[:, :, 1:D + 1, 1:H + 1, 0] = 2 * phi[..., 0] - phi[..., 1]
    P[:, :, 1:D + 1, 1:H + 1, W + 1] = 2 * phi[..., -1] - phi[..., -2]
    P[:, :, 1:D + 1, 0] = 2 * P[:, :, 1:D + 1, 1] - P[:, :, 1:D + 1, 2]
    P[:, :, 1:D + 1, H + 1] = 2 * P[:, :, 1:D + 1, H] - P[:, :, 1:D + 1, H - 1]
    P[:, :, 0] = 2 * P[:, :, 1] - P[:, :, 2]
    P[:, :, D + 1] = 2 * P[:, :, D] - P[:, :, D - 1]
    P16 = P.astype(np.float16)
    slabs = []
    for b in range(B):
        for q in range(4):
            # slot s, j: plane 40q-1+j -> padded idx 40q+j; row 64s-1+r -> 64s+r
            blocks = [P16[b, :, QP * q:QP * q + SLOT, RS * s:RS * s + RSTORE, :]
                      for s in range(NSLOT)]
            slab = np.concatenate(blocks, axis=1)  # [C, 126, 66, 162]
            slabs.append(np.ascontiguousarray(slab))
    return slabs


def _valid_mask():
    j = np.arange(NPART) % SLOT
    return (j >= 1) & (j <= SLOT - 2)


def kernel(displacement_field: np.ndarray) -> np.ndarray:
    from concourse.bass_utils import run_bass_kernel_spmd

    if 'nc' not in _prog_cache:
        _prog_cache['nc'] = _build_program()
    nc = _prog_cache['nc']

    slabs = _make_slabs(displacement_field)
    band = _make_band()
    in_maps = [{"slab": s, "band": band} for s in slabs]
    res = run_bass_kernel_spmd(nc, in_maps, core_ids=list(range(N_CORES)))

    mask = _valid_mask()
    total = 0.0
    for k in range(N_CORES):
        acc = res.results[k]["acc"]          # [126, NCHUNK] f32
        total += acc[mask].sum(dtype=np.float64)
    loss = total / float(B * D * H * W)
    return np.float32(loss)


if __name__ == "__main__":
    u = np.load('/root/problem/u_input.npy')
    print("loss:", kernel(u))


# revision 11
# speedup vs baseline: 1.0817x; 1.0817x over previous
"""JacobianDeterminantLoss Trainium2 kernel (8-core SPMD).

Math: u [2,3,160,192,160] f32 -> loss = mean(relu(-det(J))) where
J = I + grad(phi), phi_c = u_c * (dim_c-1)/2, gradients np.gradient
style (central interior, one-sided edges; ghosts 2a-b make both
uniform central diffs).

Layout (per core): core = (batch b, D-quarter q). Partitions =
3 H-slots x 42 planes (40 real + 1 halo each side) = 126. Per
partition free = 66 stored h-rows (64 real + halo) x 162 cols
(160 + ghost) fp16.

Host folds everything linear into the input: phi' = u*scale/2 +
0.5*(axis_idx - center) per channel. The central diff of the ramp
is exactly the +1 diagonal of J, so the device never adds 1.

Device per 12-row chunk:
- PE: block-diag band matmul -> D-diffs A,d,g in PSUM f32.
- ACT: PSUM -> SBUF fp16 copies; final relu + free-dim accum.
- DVE: shifted-diff subs + cofactor products (fp16 2x mode).
- Pool(GPSIMD): 2 diffs + the 3 (A,d,g)*minor products as
  scalar_tensor_tensor (its cheapest op class in the cost model).
Host: mask halo partitions, sum, divide by N.
"""
import sys
import numpy as np

if '/opt/trn_rl_repo' not in sys.path:
    sys.path.insert(0, '/opt/trn_rl_repo')

B, C, D, H, W = 2, 3, 160, 192, 160
N_CORES = 8
QP = D // 4                  # 40 planes per quarter
SLOT = QP + 2                # 42 partitions per slot
NSLOT = 3
NPART = NSLOT * SLOT         # 126
RS = H // NSLOT              # 64 real rows per slot
RSTORE = RS + 2              # 66 stored rows
WG = W + 2                   # 162 stored cols
CHUNKS = [(0, 4), (4, 12), (16, 12), (28, 12), (40, 12), (52, 8), (60, 4)]
NCHUNK = len(CHUNKS)
# DMA row sections (cover chunk c + halo before chunk c computes)
SECS = [(0, 6), (6, 12), (18, 12), (30, 12), (42, 12), (54, 12)]
# chunks whose M1 f*h product runs on Pool instead of DVE (balance knob)
POOL_EXTRA = set()

_prog_cache = {}


def _build_program():
    import concourse.tile as tile
    import concourse.mybir as mybir
    from concourse import bacc

    fp16 = mybir.dt.float16
    f32 = mybir.dt.float32
    AT = mybir.AluOpType
    AF = mybir.ActivationFunctionType

    nc = bacc.Bacc("TRN2", target_bir_lowering=False, debug=False,
                   num_devices=N_CORES)
    slab_in = nc.dram_tensor("slab", [C, NPART, RSTORE, WG], fp16,
                             kind="ExternalInput")
    band_in = nc.dram_tensor("band", [128, 128], fp16, kind="ExternalInput")
    acc_out = nc.dram_tensor("acc", [NPART, NCHUNK], f32,
                             kind="ExternalOutput")

    with tile.TileContext(nc) as tc:
        with tc.tile_pool(name="inp", bufs=1) as inp, \
             tc.tile_pool(name="piece", bufs=2) as piece, \
             tc.tile_pool(name="dveonly", bufs=1) as dv, \
             tc.tile_pool(name="cross", bufs=2) as cx, \
             tc.tile_pool(name="misc", bufs=1) as misc, \
             tc.tile_pool(name="psum", bufs=1, space="PSUM") as psum:
            band = misc.tile([128, 128], fp16)
            nc.sync.dma_start(band[:], band_in[:])
            acc_sb = misc.tile([128, NCHUNK], f32)

            XYZ = []
            for c in range(C):
                t = inp.tile([128, RSTORE, WG], fp16, tag=f"in{c}")
                XYZ.append(t)
            for (s0, sn) in SECS:
                for c in range(C):
                    nc.sync.dma_start(XYZ[c][0:NPART, s0:s0 + sn],
                                      slab_in[c, :, s0:s0 + sn])
            X, Y, Z = XYZ

            def stage1(ci):
                """PE D-diffs + ACT copies + diffs/products/minors."""
                r0, nr = CHUNKS[ci]
                pcs = []
                for ch in range(C):
                    pc = piece.tile([128, 12, W], fp16, tag=f"pc{ch}",
                                    name=f"pc{ch}")
                    for g0 in range(0, nr, 6):
                        gn = min(6, nr - g0)
                        hb = (gn + 1) // 2
                        ps = psum.tile([128, 2, 512], f32, tag=f"ps{ch}",
                                       name=f"ps{ch}")
                        for hh in range(2):
                            rr = g0 + hb * hh
                            rows = min(hb, gn - hb * hh)
                            if rows <= 0:
                                continue
                            nc.tensor.matmul(
                                ps[0:NPART, hh, 0:rows * W],
                                band[0:NPART, 0:NPART],
                                XYZ[ch][0:NPART, 1 + r0 + rr:1 + r0 + rr + rows,
                                        1:1 + W],
                                start=True, stop=True)
                        nc.scalar.copy(pc[0:NPART, g0:g0 + gn, :],
                                       ps[0:NPART, :, 0:hb * W])
                    pcs.append(pc)

                def hv(t, dr):
                    return t[0:NPART, r0 + 1 + dr:r0 + 1 + dr + nr, 1:1 + W]

                def wv(t, dw):
                    return t[0:NPART, r0 + 1:r0 + 1 + nr, 1 + dw:1 + dw + W]

                def dtile(tag):
                    return dv.tile([128, 12, W], fp16, tag=tag, name=tag)

                def ctile(tag):
                    return cx.tile([128, 12, W], fp16, tag=tag, name=tag)

                def vb(t):
                    return t[0:NPART, 0:nr]

                # diffs: b on Pool; c,E,f,h_,I on DVE
                b_ = ctile("b")
                nc.gpsimd.tensor_sub(vb(b_), hv(X, 1), hv(X, -1))
                c_ = dtile("c")
                nc.vector.tensor_sub(vb(c_), wv(X, 1), wv(X, -1))
                E_ = dtile("E")
                nc.vector.tensor_sub(vb(E_), hv(Y, 1), hv(Y, -1))
                f_ = dtile("f")
                nc.vector.tensor_sub(vb(f_), wv(Y, 1), wv(Y, -1))
                h_ = dtile("h")
                nc.vector.tensor_sub(vb(h_), hv(Z, 1), hv(Z, -1))
                I_ = dtile("i")
                nc.vector.tensor_sub(vb(I_), wv(Z, 1), wv(Z, -1))

                # det = A*(EI - fh) - d*(bI - ch) + g*(bf - cE)
                p1 = dtile("p1")
                p2 = ctile("p2") if ci in POOL_EXTRA else dtile("p2")
                M1 = ctile("M1")
                nc.vector.tensor_mul(vb(p1), vb(E_), vb(I_))
                if ci in POOL_EXTRA:
                    nc.gpsimd.tensor_mul(vb(p2), vb(f_), vb(h_))
                else:
                    nc.vector.tensor_mul(vb(p2), vb(f_), vb(h_))
                nc.vector.tensor_sub(vb(M1), vb(p1), vb(p2))
                M2 = ctile("M2")
                nc.vector.tensor_mul(vb(p1), vb(b_), vb(I_))
                nc.vector.tensor_mul(vb(p2), vb(c_), vb(h_))
                nc.vector.tensor_sub(vb(M2), vb(p1), vb(p2))
                M3 = ctile("M3")
                nc.vector.tensor_mul(vb(p1), vb(b_), vb(f_))
                nc.vector.tensor_mul(vb(p2), vb(c_), vb(E_))
                nc.vector.tensor_sub(vb(M3), vb(p1), vb(p2))
                return pcs, (M1, M2, M3)

            def stage2(ci, pcs, Ms):
                r0, nr = CHUNKS[ci]
                A_, d_, g_ = pcs
                M1, M2, M3 = Ms

                def dtile(tag):
                    return dv.tile([128, 12, W], fp16, tag=tag, name=tag)

                def ctile(tag):
                    return cx.tile([128, 12, W], fp16, tag=tag, name=tag)

                def vb(t):
                    return t[0:NPART, 0:nr]

                T1 = ctile("T1")
                nc.gpsimd.tensor_mul(vb(T1), A_[0:NPART, 0:nr], vb(M1))
                T2 = ctile("T2")
                nc.gpsimd.tensor_mul(vb(T2), d_[0:NPART, 0:nr], vb(M2))
                T3 = ctile("T3")
                nc.gpsimd.tensor_mul(vb(T3), g_[0:NPART, 0:nr], vb(M3))
                n1 = dtile("n1")
                nc.vector.tensor_sub(vb(n1), vb(T2), vb(T1))
                nd = ctile("nd")
                nc.vector.tensor_sub(vb(nd), vb(n1), vb(T3))
                trash = ctile("trash")
                nc.scalar.activation(vb(trash), vb(nd), AF.Relu,
                                     accum_out=acc_sb[0:NPART, ci:ci + 1])

            pending = None
            for ci in range(NCHUNK):
                s1 = stage1(ci)
                if pending is not None:
                    stage2(ci - 1, *pending)
                pending = s1
            nc.sync.dma_start(acc_out[:, 0:NCHUNK - 1],
                              acc_sb[0:NPART, 0:NCHUNK - 1])
            stage2(NCHUNK - 1, *pending)
            nc.sync.dma_start(acc_out[:, NCHUNK - 1:NCHUNK],
                              acc_sb[0:NPART, NCHUNK - 1:NCHUNK])
    nc.compile()
    return nc


def _make_band():
    band = np.zeros((128, 128), dtype=np.float16)
    for p in range(NPART):
        j = p % SLOT
        if j <= SLOT - 2:
            band[p + 1, p] = 1.0
        if j >= 1:
            band[p - 1, p] = -1.0
    return band


def _make_slabs(u):
    """u [2,3,160,192,160] -> 8 per-core slabs [3, 126, 66, 162] fp16."""
    u = np.asarray(u, dtype=np.float32)
    sc = np.array([(D - 1) / 4.0, (H - 1) / 4.0, (W - 1) / 4.0],
                  dtype=np.float32)
    phi = u * sc[None, :, None, None, None]
    # +1 diagonal as linear ramps (centered to limit fp16 magnitude)
    rd = 0.5 * (np.arange(D, dtype=np.float32) - (D - 1) / 2.0)
    rh = 0.5 * (np.arange(H, dtype=np.float32) - (H - 1) / 2.0)
    rw = 0.5 * (np.arange(W, dtype=np.float32) - (W - 1) / 2.0)
    phi[:, 0] += rd[:, None, None]
    phi[:, 1] += rh[None, :, None]
    phi[:, 2] += rw[None, None, :]
    # pad with linear-extrapolation ghosts on all three axes
    P = np.empty((B, C, D + 2, H + 2, W + 2), dtype=np.float32)
    P[:, :, 1:D + 1, 1:H + 1, 1:W + 1] = phi
    P[:, :, 1:D + 1, 1:H + 1, 0] = 2 * phi[..., 0] - phi[..., 1]
    P[:, :, 1:D + 1, 1:H + 1, W + 1] = 2 * phi[..., -1] - phi[..., -2]
    P[:, :, 1:D + 1, 0] = 2 * P[:, :, 1:D + 1, 1] - P[:, :, 1:D + 1, 2]
    P[:, :, 1:D + 1, H + 1] = 2 * P[:, :, 1:D + 1, H] - P[:, :, 1:D + 1, H - 1]
    P[:, :, 0] = 2 * P[:, :, 1] - P[:, :, 2]
    P[:, :, D + 1] = 2 * P[:, :, D] - P[:, :, D - 1]
    P16 = P.astype(np.float16)
    slabs = []
    for b in range(B):
        for q in range(4):
            # slot s, j: plane 40q-1+j -> padded idx 40q+j; row 64s-1+r -> 64s+r
            blocks = [P16[b, :, QP * q:QP * q + SLOT, RS * s:RS * s + RSTORE, :]
                      for s in range(NSLOT)]
            slab = np.concatenate(blocks, axis=1)  # [C, 126, 66, 162]
            slabs.append(np.ascontiguousarray(slab))
    return slabs


def _valid_mask():
    j = np.arange(NPART) % SLOT
    return (j >= 1) & (j <= SLOT - 2)


def kernel(displacement_field: np.ndarray) -> np.ndarray:
    from concourse.bass_utils import run_bass_kernel_spmd

    if 'nc' not in _prog_cache:
        _prog_cache['nc'] = _build_program()
    nc = _prog_cache['nc']

    slabs = _make_slabs(displacement_field)
    band = _make_band()
    in_maps = [{"slab": s, "band": band} for s in slabs]
    res = run_bass_kernel_spmd(nc, in_maps, core_ids=list(range(N_CORES)))

    mask = _valid_mask()
    total = 0.0
    for k in range(N_CORES):
        acc = res.results[k]["acc"]          # [126, NCHUNK] f32
        total += acc[mask].sum(dtype=np.float64)
    loss = total / float(B * D * H * W)
    return np.float32(loss)


if __name__ == "__main__":
    u = np.load('/root/problem/u_input.npy')
    print("loss:", kernel(u))


# revision 13
# speedup vs baseline: 1.0990x; 1.0161x over previous
"""JacobianDeterminantLoss Trainium2 kernel (8-core SPMD).

Math: u [2,3,160,192,160] f32 -> loss = mean(relu(-det(J))) where
J = I + grad(phi), phi_c = u_c * (dim_c-1)/2, gradients np.gradient
style (central interior, one-sided edges; ghosts 2a-b make both
uniform central diffs).

Layout (per core): core = (batch b, D-quarter q). Partitions =
3 H-slots x 42 planes (40 real + 1 halo each side) = 126. Per
partition free = 66 stored h-rows (64 real + halo) x 162 cols
(160 + ghost) fp16.

Host folds everything linear into the input: phi' = u*scale/2 +
0.5*(axis_idx - center) per channel. The central diff of the ramp
is exactly the +1 diagonal of J, so the device never adds 1.

Device per 12-row chunk:
- PE: block-diag band matmul -> D-diffs A,d,g in PSUM f32.
- ACT: PSUM -> SBUF fp16 copies; final relu + free-dim accum.
- DVE: shifted-diff subs + cofactor products (fp16 2x mode).
- Pool(GPSIMD): 2 diffs + the 3 (A,d,g)*minor products as
  scalar_tensor_tensor (its cheapest op class in the cost model).
Host: mask halo partitions, sum, divide by N.
"""
import sys
import numpy as np

if '/opt/trn_rl_repo' not in sys.path:
    sys.path.insert(0, '/opt/trn_rl_repo')

B, C, D, H, W = 2, 3, 160, 192, 160
N_CORES = 8
QP = D // 4                  # 40 planes per quarter
SLOT = QP + 2                # 42 partitions per slot
NSLOT = 3
NPART = NSLOT * SLOT         # 126
RS = H // NSLOT              # 64 real rows per slot
RSTORE = RS + 2              # 66 stored rows
WG = W + 2                   # 162 stored cols
CHUNKS = [(0, 4), (4, 12), (16, 12), (28, 12), (40, 12), (52, 8), (60, 4)]
NCHUNK = len(CHUNKS)
# DMA row sections (cover chunk c + halo before chunk c computes)
SECS = [(0, 6), (6, 12), (18, 12), (30, 12), (42, 12), (54, 12)]
# chunks whose M1 f*h product runs on Pool instead of DVE (balance knob)
POOL_EXTRA = set()

_prog_cache = {}


def _build_program():
    import concourse.tile as tile
    import concourse.mybir as mybir
    from concourse import bacc

    fp16 = mybir.dt.float16
    f32 = mybir.dt.float32
    AT = mybir.AluOpType
    AF = mybir.ActivationFunctionType

    nc = bacc.Bacc("TRN2", target_bir_lowering=False, debug=False,
                   num_devices=N_CORES)
    slab_in = nc.dram_tensor("slab", [C, NPART, RSTORE, WG], fp16,
                             kind="ExternalInput")
    band_in = nc.dram_tensor("band", [128, 128], fp16, kind="ExternalInput")
    acc_out = nc.dram_tensor("acc", [NPART, NCHUNK], f32,
                             kind="ExternalOutput")

    with tile.TileContext(nc) as tc:
        with tc.tile_pool(name="inp", bufs=1) as inp, \
             tc.tile_pool(name="piece", bufs=2) as piece, \
             tc.tile_pool(name="dveonly", bufs=1) as dv, \
             tc.tile_pool(name="cross", bufs=2) as cx, \
             tc.tile_pool(name="misc", bufs=1) as misc, \
             tc.tile_pool(name="psum", bufs=1, space="PSUM") as psum:
            band = misc.tile([128, 128], fp16)
            acc_sb = misc.tile([128, NCHUNK], f32)

            XYZ = []
            for c in range(C):
                t = inp.tile([128, RSTORE, WG], fp16, tag=f"in{c}")
                XYZ.append(t)
            for si, (s0, sn) in enumerate(SECS):
                for c in range(C):
                    nc.sync.dma_start(XYZ[c][0:NPART, s0:s0 + sn],
                                      slab_in[c, :, s0:s0 + sn])
                if si == 0:
                    nc.sync.dma_start(band[:], band_in[:])
            X, Y, Z = XYZ

            def stage1(ci):
                """PE D-diffs + ACT copies + diffs/products/minors."""
                r0, nr = CHUNKS[ci]
                pcs = []
                for ch in range(C):
                    pc = piece.tile([128, 12, W], fp16, tag=f"pc{ch}",
                                    name=f"pc{ch}")
                    for g0 in range(0, nr, 6):
                        gn = min(6, nr - g0)
                        hb = (gn + 1) // 2
                        ps = psum.tile([128, 2, 512], f32, tag=f"ps{ch}",
                                       name=f"ps{ch}")
                        for hh in range(2):
                            rr = g0 + hb * hh
                            rows = min(hb, gn - hb * hh)
                            if rows <= 0:
                                continue
                            nc.tensor.matmul(
                                ps[0:NPART, hh, 0:rows * W],
                                band[0:NPART, 0:NPART],
                                XYZ[ch][0:NPART, 1 + r0 + rr:1 + r0 + rr + rows,
                                        1:1 + W],
                                start=True, stop=True)
                        nc.scalar.copy(pc[0:NPART, g0:g0 + gn, :],
                                       ps[0:NPART, :, 0:hb * W])
                    pcs.append(pc)

                def hv(t, dr):
                    return t[0:NPART, r0 + 1 + dr:r0 + 1 + dr + nr, 1:1 + W]

                def wv(t, dw):
                    return t[0:NPART, r0 + 1:r0 + 1 + nr, 1 + dw:1 + dw + W]

                def dtile(tag):
                    return dv.tile([128, 12, W], fp16, tag=tag, name=tag)

                def ctile(tag):
                    return cx.tile([128, 12, W], fp16, tag=tag, name=tag)

                def vb(t):
                    return t[0:NPART, 0:nr]

                # diffs: b on Pool; c,E,f,h_,I on DVE
                b_ = ctile("b")
                nc.gpsimd.tensor_sub(vb(b_), hv(X, 1), hv(X, -1))
                c_ = dtile("c")
                nc.vector.tensor_sub(vb(c_), wv(X, 1), wv(X, -1))
                E_ = dtile("E")
                nc.vector.tensor_sub(vb(E_), hv(Y, 1), hv(Y, -1))
                f_ = dtile("f")
                nc.vector.tensor_sub(vb(f_), wv(Y, 1), wv(Y, -1))
                h_ = dtile("h")
                nc.vector.tensor_sub(vb(h_), hv(Z, 1), hv(Z, -1))
                I_ = dtile("i")
                nc.vector.tensor_sub(vb(I_), wv(Z, 1), wv(Z, -1))

                # det = A*(EI - fh) - d*(bI - ch) + g*(bf - cE)
                p1 = dtile("p1")
                p2 = ctile("p2") if ci in POOL_EXTRA else dtile("p2")
                M1 = ctile("M1")
                nc.vector.tensor_mul(vb(p1), vb(E_), vb(I_))
                if ci in POOL_EXTRA:
                    nc.gpsimd.tensor_mul(vb(p2), vb(f_), vb(h_))
                else:
                    nc.vector.tensor_mul(vb(p2), vb(f_), vb(h_))
                nc.vector.tensor_sub(vb(M1), vb(p1), vb(p2))
                M2 = ctile("M2")
                nc.vector.tensor_mul(vb(p1), vb(b_), vb(I_))
                nc.vector.tensor_mul(vb(p2), vb(c_), vb(h_))
                nc.vector.tensor_sub(vb(M2), vb(p1), vb(p2))
                M3 = ctile("M3")
                nc.vector.tensor_mul(vb(p1), vb(b_), vb(f_))
                nc.vector.tensor_mul(vb(p2), vb(c_), vb(E_))
                nc.vector.tensor_sub(vb(M3), vb(p1), vb(p2))
                return pcs, (M1, M2, M3)

            def stage2(ci, pcs, Ms):
                r0, nr = CHUNKS[ci]
                A_, d_, g_ = pcs
                M1, M2, M3 = Ms

                def dtile(tag):
                    return dv.tile([128, 12, W], fp16, tag=tag, name=tag)

                def ctile(tag):
                    return cx.tile([128, 12, W], fp16, tag=tag, name=tag)

                def vb(t):
                    return t[0:NPART, 0:nr]

                teng = nc.vector if ci == NCHUNK - 1 else nc.gpsimd
                T1 = ctile("T1")
                teng.tensor_mul(vb(T1), A_[0:NPART, 0:nr], vb(M1))
                T2 = ctile("T2")
                teng.tensor_mul(vb(T2), d_[0:NPART, 0:nr], vb(M2))
                T3 = ctile("T3")
                teng.tensor_mul(vb(T3), g_[0:NPART, 0:nr], vb(M3))
                n1 = dtile("n1")
                nc.vector.tensor_sub(vb(n1), vb(T2), vb(T1))
                nd = ctile("nd")
                nc.vector.tensor_sub(vb(nd), vb(n1), vb(T3))
                trash = ctile("trash")
                nc.scalar.activation(vb(trash), vb(nd), AF.Relu,
                                     accum_out=acc_sb[0:NPART, ci:ci + 1])

            pending = None
            for ci in range(NCHUNK):
                s1 = stage1(ci)
                if pending is not None:
                    stage2(ci - 1, *pending)
                pending = s1
            nc.sync.dma_start(acc_out[:, 0:NCHUNK - 1],
                              acc_sb[0:NPART, 0:NCHUNK - 1])
            stage2(NCHUNK - 1, *pending)
            nc.sync.dma_start(acc_out[:, NCHUNK - 1:NCHUNK],
                              acc_sb[0:NPART, NCHUNK - 1:NCHUNK])
    nc.compile()
    return nc


def _make_band():
    band = np.zeros((128, 128), dtype=np.float16)
    for p in range(NPART):
        j = p % SLOT
        if j <= SLOT - 2:
            band[p + 1, p] = 1.0
        if j >= 1:
            band[p - 1, p] = -1.0
    return band


def _make_slabs(u):
    """u [2,3,160,192,160] -> 8 per-core slabs [3, 126, 66, 162] fp16."""
    u = np.asarray(u, dtype=np.float32)
    sc = np.array([(D - 1) / 4.0, (H - 1) / 4.0, (W - 1) / 4.0],
                  dtype=np.float32)
    phi = u * sc[None, :, None, None, None]
    # +1 diagonal as linear ramps (centered to limit fp16 magnitude)
    rd = 0.5 * (np.arange(D, dtype=np.float32) - (D - 1) / 2.0)
    rh = 0.5 * (np.arange(H, dtype=np.float32) - (H - 1) / 2.0)
    rw = 0.5 * (np.arange(W, dtype=np.float32) - (W - 1) / 2.0)
    phi[:, 0] += rd[:, None, None]
    phi[:, 1] += rh[None, :, None]
    phi[:, 2] += rw[None, None, :]
    # pad with linear-extrapolation ghosts on all three axes
    P = np.empty((B, C, D + 2, H + 2, W + 2), dtype=np.float32)
    P[:, :, 1:D + 1, 1:H + 1, 1:W + 1] = phi
    P[:, :, 1:D + 1, 1:H + 1, 0] = 2 * phi[..., 0] - phi[..., 1]
    P[:, :, 1:D + 1, 1:H + 1, W + 1] = 2 * phi[..., -1] - phi[..., -2]
    P[:, :, 1:D + 1, 0] = 2 * P[:, :, 1:D + 1, 1] - P[:, :, 1:D + 1, 2]
    P[:, :, 1:D + 1, H + 1] = 2 * P[:, :, 1:D + 1, H] - P[:, :, 1:D + 1, H - 1]
    P[:, :, 0] = 2 * P[:, :, 1] - P[:, :, 2]
    P[:, :, D + 1] = 2 * P[:, :, D] - P[:, :, D - 1]
    P16 = P.astype(np.float16)
    slabs = []
    for b in range(B):
        for q in range(4):
            # slot s, j: plane 40q-1+j -> padded idx 40q+j; row 64s-1+r -> 64s+r
            blocks = [P16[b, :, QP * q:QP * q + SLOT, RS * s:RS * s + RSTORE, :]
                      for s in range(NSLOT)]
            slab = np.concatenate(blocks, axis=1)  # [C, 126, 66, 162]
            slabs.append(np.ascontiguousarray(slab))
    return slabs


def _valid_mask():
    j = np.arange(NPART) % SLOT
    return (j >= 1) & (j <= SLOT - 2)


def kernel(displacement_field: np.ndarray) -> np.ndarray:
    from concourse.bass_utils import run_bass_kernel_spmd

    if 'nc' not in _prog_cache:
        _prog_cache['nc'] = _build_program()
    nc = _prog_cache['nc']

    slabs = _make_slabs(displacement_field)
    band = _make_band()
    in_maps = [{"slab": s, "band": band} for s in slabs]
    res = run_bass_kernel_spmd(nc, in_maps, core_ids=list(range(N_CORES)))

    mask = _valid_mask()
    total = 0.0
    for k in range(N_CORES):
        acc = res.results[k]["acc"]          # [126, NCHUNK] f32
        total += acc[mask].sum(dtype=np.float64)
    loss = total / float(B * D * H * W)
    return np.float32(loss)


if __name__ == "__main__":
    u = np.load('/root/problem/u_input.npy')
    print("loss:", kernel(u))
